# revision 9
# baseline (speedup 1.0000x reference)
"""ConstituencyAwareAttention Trainium2 kernel (v3).

Strategy: pure data parallelism -- B=8 batch elements across 8 NeuronCores,
one full attention problem per core (S=1024, H=1024, nh=16, hd=64).

v3 restructure (vs v2 baseline at ~283us):
  * Host-side fp16 pre-layout of X / Wq / Wk / Wv in the exact SBUF tiling:
    halves input DMA bytes (16.8 -> 8.4 MB), gives 2KB-contiguous DMA
    segments, and removes all on-device weight/X casts from the vector
    engine (-34us DVE).  First matmul moves from ~38us to ~11us.
  * PE warm-up chain (32 fp16 junk matmuls) during the DMA head so the
    HAM clock gate reaches 8/8 before the first real projection.
  * Scores emitted qc-outer / half-inner: consecutive matmuls alternate
    row groups (tile_position (0,0)/(64,0)) so the two K=64 matmuls run
    CONCURRENTLY in the PE array (~2x scores throughput).
  * Constituency penalty fix as one masked tensor_mul per (half, kt)
    against a precomputed [128,128] mask (was 2 tensor_scalars).
  * V projection rebalanced: heads 0-7 in slot 0, heads 8-15 spread over
    slots 1-4 (slot 0 was 2x the PE work of other slots).
  * Split-chain AV epilogue: pair 7's AV contractions over kt 0..6 are
    issued before the last exps retire; only the kt=7 closers + transposes
    + ship remain after the final exp (tail 14us -> ~5us).
"""

import math
import sys

if "/opt/trn_rl_repo" not in sys.path:
    sys.path.insert(0, "/opt/trn_rl_repo")

import numpy as np

import concourse.bacc as bacc
import concourse.tile as tile
from concourse import mybir
from concourse.bass_utils import run_bass_kernel_spmd

F16 = mybir.dt.float16
F32 = mybir.dt.float32

B, S, H = 8, 1024, 1024
NH, HD = 16, 64
P = 128
SO = S // P   # 8 S-chunks
KO = H // P   # 8 contraction chunks
NP = NH // 2  # 8 head pairs
PEN = 0.5
FIX = float(math.exp(PEN))
SCALE = 1.0 / math.sqrt(HD)

_programs = {}


def _build_program(with_bv: bool):
    nc = bacc.Bacc("TRN2", target_bir_lowering=False, debug=False)

    # Host pre-laid-out fp16 inputs (see _in_maps for the exact packing).
    xt = nc.dram_tensor("xt", [P, KO, S], F16, kind="ExternalInput").ap()
    wq = nc.dram_tensor("wq", [P, NP, KO, P], F16, kind="ExternalInput").ap()
    wk = nc.dram_tensor("wk", [P, NP, KO, P], F16, kind="ExternalInput").ap()
    wv = nc.dram_tensor("wv", [P, KO, H], F16, kind="ExternalInput").ap()
    bq = nc.dram_tensor("bq", [H], F32, kind="ExternalInput").ap()
    bk = nc.dram_tensor("bk", [H], F32, kind="ExternalInput").ap()
    bv = nc.dram_tensor("bv", [H], F32, kind="ExternalInput").ap()
    out = nc.dram_tensor("out", [S, H], F32, kind="ExternalOutput").ap()

    Exp = mybir.ActivationFunctionType.Exp

    out_r = out.rearrange("(o p) n -> p o n", p=P)         # [128, SO, H]

    with tile.TileContext(nc) as tc:
        with (
            tc.tile_pool(name="persist", bufs=1) as persist,
            tc.tile_pool(name="probs", bufs=1) as probs,
            tc.tile_pool(name="score_ps", bufs=1, space="PSUM") as score_ps,
            tc.tile_pool(name="proj_ps", bufs=2, space="PSUM") as proj_ps,
            tc.tile_pool(name="work_ps", bufs=2, space="PSUM") as work_ps,
        ):
            # ---------------- persistent SBUF ----------------
            XT = persist.tile([P, KO, S], F16, name="XT")
            QT3 = persist.tile([P, 3, S], F16, name="QT3")
            KT3 = persist.tile([P, 3, S], F16, name="KT3")
            VA = persist.tile([P, SO, NH * (HD + 1)], F16, name="VA")
            wqh = persist.tile([P, NP, KO, P], F16, name="wqh")
            wkh = persist.tile([P, NP, KO, P], F16, name="wkh")
            wvh = persist.tile([P, KO, H], F16, name="wvh")
            nbias = persist.tile([P, 1], F32, name="nbias")
            bq_s = persist.tile([P, KO], F32, name="bq_s")
            bk_s = persist.tile([P, KO], F32, name="bk_s")
            pmask = persist.tile([P, P], F16, name="pmask")
            w16 = persist.tile([P, P], F16, name="w16")

            # vector-engine setup (fast, unblocks warmup + exp + penalty)
            nc.vector.memset(w16[:], 1.0)
            nc.vector.memset(nbias[:], -PEN)
            nc.vector.memset(pmask[:], 1.0)
            nc.vector.memset(pmask[0:64, 0:64], FIX)
            nc.vector.memset(pmask[64:128, 64:128], FIX)
            # ones columns of V_aug only; V block copies fill the rest
            VA_v = VA[:].rearrange("p s (h c) -> p s h c", c=HD + 1)
            nc.vector.memset(VA_v[:, :, :, HD : HD + 1], 1.0)

            # ---------------- DMA: prioritized, fp16, contiguous ----------
            # 4 hw DMA queues for the head: sync (SP), scalar (Activation),
            # gpsimd, and vector (only its first-scores inputs; vector's
            # compute role starts later).  X chunks + first q/k slices
            # first; biases and Wv behind; late q/k slices last.
            nc.scalar.dma_start(out=wkh[:, 0, :, :], in_=wk[:, 0, :, :])
            nc.sync.dma_start(out=XT[:, 0:3, :], in_=xt[:, 0:3, :])
            nc.scalar.dma_start(out=XT[:, 3:6, :], in_=xt[:, 3:6, :])
            nc.gpsimd.dma_start(out=XT[:, 6:8, :], in_=xt[:, 6:8, :])
            nc.sync.dma_start(out=wqh[:, 0, :, :], in_=wq[:, 0, :, :])
            nc.gpsimd.dma_start(out=wqh[:, 1, :, :], in_=wq[:, 1, :, :])
            nc.gpsimd.dma_start(out=wkh[:, 1, :, :], in_=wk[:, 1, :, :])
            nc.sync.dma_start(out=bq_s[:], in_=bq.rearrange("(o p) -> p o", p=P))
            nc.sync.dma_start(out=bk_s[:], in_=bk.rearrange("(o p) -> p o", p=P))
            nc.vector.tensor_scalar_mul(bq_s[:], bq_s[:], SCALE)
            nc.scalar.dma_start(out=wvh[:, 0:4, :], in_=wv[:, 0:4, :])
            nc.sync.dma_start(out=wvh[:, 4:8, :], in_=wv[:, 4:8, :])
            for mo in range(2, NP):
                nc.gpsimd.dma_start(out=wqh[:, mo, :, :], in_=wq[:, mo, :, :])
                nc.gpsimd.dma_start(out=wkh[:, mo, :, :], in_=wk[:, mo, :, :])

            # ---------------- PE warm-up ----------------
            # ~48 junk matmuls (N=128, fp16) keep the PE busy from ~7us so
            # the HAM clock-gate is at 8/8 when the first projection lands.
            warm = work_ps.tile([P, 512], F32, name="warm", tag="work")
            for w in range(48):
                nc.tensor.matmul(
                    warm[:, 0:P], w16[:], w16[:],
                    start=(w == 0), stop=(w == 47),
                )

            # probs rings: 2 pairs in flight (scored / being consumed by AV)
            prT = [
                [
                    probs.tile([P, KO, S], F16, name=f"prT_{h}_{r}")
                    for r in range(2)
                ]
                for h in range(2)
            ]
            ctxt_sb = [
                probs.tile([80, 512], F16, name=f"ctxt_sb{r}")
                for r in range(4)
            ]
            tpx_sb = [
                probs.tile([P, 4, 80], F16, name=f"tpx_sb{r}")
                for r in range(4)
            ]
            for r in range(4):
                nc.vector.memset(ctxt_sb[r][HD : 80, :], 0.0)
            inv_sb = [
                probs.tile([P, 4], F32, name=f"inv{r}")
                for r in range(4)
            ]
            out_pair = [
                probs.tile([P, SO, P], F32, name=f"out_pair{r}")
                for r in range(2)
            ]

            # score psum: one [128, S] tile per half, reused across kt
            pst = [
                score_ps.tile([P, S], F32, name=f"pst{h}", tag=f"pst{h}")
                for h in range(2)
            ]

            # ---------------- emission helpers ----------------
            def proj_burst(mo, which, sc):
                """One Q or K projection burst: 8 chained MMs -> 1 psum bank,
                evacuated to the QT/KT ring with scale+bias fused."""
                wsb = wqh if which == "q" else wkh
                r = mo % 3
                ps = proj_ps.tile([P, 512], F32, name="ps", tag="proj")
                for kh in range(KO):
                    nc.tensor.matmul(
                        ps[:],
                        wsb[:, mo, kh, :],
                        XT[:, kh, sc * 512 : (sc + 1) * 512],
                        start=(kh == 0),
                        stop=(kh == KO - 1),
                    )
                if which == "q":
                    nc.vector.tensor_scalar(
                        QT3[:, r, sc * 512 : (sc + 1) * 512], ps[:],
                        SCALE, bq_s[:, mo : mo + 1],
                        mybir.AluOpType.mult, mybir.AluOpType.add,
                    )
                else:
                    nc.vector.tensor_scalar_add(
                        KT3[:, r, sc * 512 : (sc + 1) * 512], ps[:],
                        bk_s[:, mo : mo + 1],
                    )

            def v_burst(so, ncol):
                """V projection burst: 8 chained MMs -> work ring bank, then
                strided copy into V_aug (leaving the ones columns alone)."""
                ps = work_ps.tile([P, 512], F32, name="vps", tag="work")
                for kh in range(KO):
                    nc.tensor.matmul(
                        ps[:],
                        XT[:, kh, so * P : (so + 1) * P],
                        wvh[:, kh, ncol * 512 : (ncol + 1) * 512],
                        start=(kh == 0),
                        stop=(kh == KO - 1),
                    )
                va_v = VA[:, so, :].rearrange("p (h c) -> p h c", c=HD + 1)
                nc.vector.tensor_copy(
                    va_v[:, ncol * 8 : (ncol + 1) * 8, 0:HD],
                    ps[:].rearrange("p (h c) -> p h c", c=HD),
                )

            def score_qc(i, kt, qc):
                """One qc's pair of K=64 row-tiled score MMs (concurrent)."""
                r = i % 3
                for half in range(2):
                    lo = half * 64
                    nc.tensor.matmul(
                        pst[half][:, qc * 512 : (qc + 1) * 512],
                        KT3[lo : lo + 64, r, kt * P : (kt + 1) * P],
                        QT3[lo : lo + 64, r, qc * 512 : (qc + 1) * 512],
                        start=True,
                        stop=True,
                        tile_position=(lo, 0),
                    )

            def exp_kt(i, kt, ring):
                """Per-(half, qc) exps: fine-grained pst release so the next
                kt's score pairs become concurrently ready; then penalty."""
                for qc in range(2):
                    for half in range(2):
                        dst = prT[half][ring]
                        nc.scalar.activation(
                            dst[:, kt, qc * 512 : (qc + 1) * 512],
                            pst[half][:, qc * 512 : (qc + 1) * 512],
                            Exp, bias=nbias[:],
                        )
                for half in range(2):
                    dst = prT[half][ring]
                    nc.vector.tensor_mul(
                        dst[:, kt, kt * P : (kt + 1) * P],
                        dst[:, kt, kt * P : (kt + 1) * P],
                        pmask[:],
                    )

            def av_open(i, half, qc, ring, pool, kt_hi):
                """Open an AV accumulation chain over kt 0..kt_hi-1."""
                h = 2 * i + half
                ctx = pool.tile([P, 512], F32, name="ctx",
                                tag="proj" if pool is proj_ps else "work")
                for kt in range(kt_hi):
                    nc.tensor.matmul(
                        ctx[0 : HD + 1, :],
                        VA[:, kt, h * (HD + 1) : (h + 1) * (HD + 1)],
                        prT[half][ring][:, kt, qc * 512 : (qc + 1) * 512],
                        start=(kt == 0),
                        stop=False,
                    )
                return ctx

            def av_close(ctx, i, half, qc, ring, slot4, kt_lo, eng_scalar=False):
                """Close an AV chain (kt_lo..7) and evacuate to SBUF."""
                h = 2 * i + half
                for kt in range(kt_lo, KO):
                    nc.tensor.matmul(
                        ctx[0 : HD + 1, :],
                        VA[:, kt, h * (HD + 1) : (h + 1) * (HD + 1)],
                        prT[half][ring][:, kt, qc * 512 : (qc + 1) * 512],
                        start=False,
                        stop=(kt == KO - 1),
                    )
                if eng_scalar:
                    nc.scalar.copy(ctxt_sb[slot4][0 : HD + 1, :], ctx[0 : HD + 1, :])
                else:
                    nc.vector.tensor_copy(
                        ctxt_sb[slot4][0 : HD + 1, :], ctx[0 : HD + 1, :]
                    )

            def av_burst(i, half, qc, ring, slot4):
                """Full AV for (pair i, head-half, q-chunk qc)."""
                ctx = av_open(i, half, qc, ring, work_ps, KO - 1)
                av_close(ctx, i, half, qc, ring, slot4, KO - 1)

            def tp_burst(half, qc, slot4, opr, pool=None, eng=None):
                """X-bar DMA-transpose ctx^T back to [q, d] + normalize."""
                tpx = tpx_sb[slot4]
                (eng or nc.sync).dma_start(out=tpx[:], in_=ctxt_sb[slot4][:],
                                           transpose=True)
                nc.vector.reciprocal(inv_sb[slot4][:], tpx[:, :, HD])
                for c4 in range(4):
                    so = qc * 4 + c4
                    nc.vector.tensor_scalar_mul(
                        out_pair[opr][:, so, half * HD : (half + 1) * HD],
                        tpx[:, c4, 0:HD],
                        inv_sb[slot4][:, c4 : c4 + 1],
                    )

            def ship_pair(i, opr):
                nc.sync.dma_start(
                    out=out_r[:, :, i * P : (i + 1) * P], in_=out_pair[opr][:]
                )

            # ---------------- bootstrap: proj(0) ----------------
            proj_burst(0, "q", 0)
            proj_burst(0, "k", 0)
            proj_burst(0, "q", 1)
            proj_burst(0, "k", 1)

            # ---------------- pair slots ----------------
            # slot i: scores(i) + exp(i) | proj(i+1) | V bursts (slot 0 +
            # spread) | i>=1: AV(i-1) + tp(i-1) + ship(i-1)
            av_seq = [(0, 0), (0, 1), (1, 0), (1, 1)]
            for i in range(NP):
                ring = i % 2
                pring = (i - 1) % 2
                opr = (i - 1) % 2

                for kt in range(KO):
                    score_qc(i, kt, 0)

                    # interleaved burst between the two score pairs
                    if i + 1 < NP:
                        if kt == 0:
                            proj_burst(i + 1, "q", 0)
                        elif kt == 2:
                            proj_burst(i + 1, "q", 1)
                        elif kt == 4:
                            proj_burst(i + 1, "k", 0)
                        elif kt == 6:
                            proj_burst(i + 1, "k", 1)
                    if i == 0:
                        if with_bv:
                            if kt < 4:
                                v_burst(2 * kt, 0)
                                v_burst(2 * kt, 1)
                            else:
                                v_burst(2 * (kt - 4) + 1, 0)
                                v_burst(2 * (kt - 4) + 1, 1)
                        else:
                            # heads 0-7 here; heads 8-15 spread over slots 1-4
                            v_burst(kt, 0)
                    elif kt % 2 == 0:
                        half, qc = av_seq[kt // 2]
                        av_burst(i - 1, half, qc, pring, kt // 2)

                    score_qc(i, kt, 1)

                    if i >= 1:
                        if kt % 2 == 1 and kt >= 3:
                            half, qc = av_seq[(kt - 3) // 2]
                            tp_burst(half, qc, (kt - 3) // 2, opr)
                        if not with_bv and 1 <= i <= 4 and kt in (1, 5):
                            v_burst(2 * (i - 1) + (0 if kt == 1 else 1), 1)

                    exp_kt(i, kt, ring)

                if i == 0 and with_bv:
                    # out += bv exactly (softmax rows sum to 1): broadcast bv
                    # across partitions via PE, add into V_aug.
                    ones1 = persist.tile([1, P], F16, name="ones1")
                    nc.vector.memset(ones1[:], 1.0)
                    bv1 = persist.tile([1, H], F16, name="bv1")
                    bv1_32 = persist.tile([1, H], F32, name="bv1_32")
                    nc.sync.dma_start(out=bv1_32[:], in_=bv[None, :])
                    nc.vector.tensor_copy(bv1[:], bv1_32[:])
                    bvb = persist.tile([P, NH * (HD + 1)], F16, name="bvb")
                    nc.vector.memset(bvb[:], 0.0)
                    bvb_v = bvb.rearrange("p (h c) -> p h c", c=HD + 1)
                    for ncol in range(2):
                        psb = proj_ps.tile([P, 512], F32, name="psb", tag="proj")
                        nc.tensor.matmul(
                            psb[:], ones1[:], bv1[:, ncol * 512 : (ncol + 1) * 512],
                            start=True, stop=True,
                        )
                        nc.vector.tensor_copy(
                            bvb_v[:, ncol * 8 : (ncol + 1) * 8, 0:HD],
                            psb[:].rearrange("p (h c) -> p h c", c=HD),
                        )
                    for so in range(SO):
                        nc.vector.tensor_add(VA[:, so, :], VA[:, so, :], bvb[:])

                if i >= 1:
                    # last tp of pair i-1 spills here; then ship the pair
                    tp_burst(1, 1, 3, opr)
                    ship_pair(i - 1, opr)

            # ---------------- epilogue: AV(7) + tp(7), split-chain --------
            # Open all four AV chains over kt 0..6 (their exps are done while
            # kt=7's exps still run); only the kt=7 closers + transposes +
            # ship are exposed after the final exp.
            ring = (NP - 1) % 2
            opr = (NP - 1) % 2
            cA0 = av_open(NP - 1, 0, 0, ring, proj_ps, KO - 1)
            cA1 = av_open(NP - 1, 0, 1, ring, proj_ps, KO - 1)
            cA2 = av_open(NP - 1, 1, 0, ring, work_ps, KO - 1)
            cA3 = av_open(NP - 1, 1, 1, ring, work_ps, KO - 1)
            av_close(cA0, NP - 1, 0, 0, ring, 0, KO - 1)
            tp_burst(0, 0, 0, opr, eng=nc.sync)
            av_close(cA1, NP - 1, 0, 1, ring, 1, KO - 1)
            tp_burst(0, 1, 1, opr, eng=nc.scalar)
            av_close(cA2, NP - 1, 1, 0, ring, 2, KO - 1)
            tp_burst(1, 0, 2, opr, eng=nc.sync)
            av_close(cA3, NP - 1, 1, 1, ring, 3, KO - 1)
            tp_burst(1, 1, 3, opr, eng=nc.scalar)
            ship_pair(NP - 1, opr)

    nc.compile()
    return nc


def _get_program(with_bv: bool):
    key = with_bv
    if key not in _programs:
        _programs[key] = _build_program(with_bv)
    return _programs[key]


def _in_maps(hidden_states, Wq, bq, Wk, bk, Wv, bv):
    # fp16 pre-layout matching the SBUF tiling (see _build_program):
    #   xt[kp, ko, s]     = X[s, ko*128+kp]
    #   wq[kp, mo, ko, c] = Wq[ko*128+kp, mo*128+c]   (wk likewise)
    #   wv[kp, ko, n]     = Wv[ko*128+kp, n]
    wq16 = np.ascontiguousarray(
        np.asarray(Wq, np.float32).astype(np.float16)
        .reshape(KO, P, NP, P).transpose(1, 2, 0, 3)
    )
    wk16 = np.ascontiguousarray(
        np.asarray(Wk, np.float32).astype(np.float16)
        .reshape(KO, P, NP, P).transpose(1, 2, 0, 3)
    )
    wv16 = np.ascontiguousarray(
        np.asarray(Wv, np.float32).astype(np.float16)
        .reshape(KO, P, H).transpose(1, 0, 2)
    )
    bq = np.ascontiguousarray(bq, np.float32)
    bk = np.ascontiguousarray(bk, np.float32)
    bv = np.ascontiguousarray(bv, np.float32)
    maps = []
    for b in range(B):
        xt16 = np.ascontiguousarray(
            np.asarray(hidden_states[b], np.float32).astype(np.float16)
            .T.reshape(KO, P, S).transpose(1, 0, 2)
        )
        maps.append({
            "xt": xt16,
            "wq": wq16, "wk": wk16, "wv": wv16,
            "bq": bq, "bk": bk, "bv": bv,
        })
    return maps


def kernel(hidden_states, Wq, bq, Wk, bk, Wv, bv):
    hidden_states = np.ascontiguousarray(hidden_states, dtype=np.float32)
    with_bv = bool(np.any(np.asarray(bv) != 0))
    nc = _get_program(with_bv)
    in_maps = _in_maps(hidden_states, Wq, bq, Wk, bk, Wv, bv)
    last_err = None
    for _attempt in range(3):
        try:
            res = run_bass_kernel_spmd(nc, in_maps, list(range(B)))
            return np.stack([res.results[b]["out"] for b in range(B)], axis=0)
        except Exception as e:  # transient NRT device errors recover on retry
            last_err = e
            import time
            time.sleep(3)
    raise last_err


# revision 11
# speedup vs baseline: 1.0142x; 1.0142x over previous
"""ConstituencyAwareAttention Trainium2 kernel (v3).

Strategy: pure data parallelism -- B=8 batch elements across 8 NeuronCores,
one full attention problem per core (S=1024, H=1024, nh=16, hd=64).

v3 restructure (vs v2 baseline at ~283us):
  * Host-side fp16 pre-layout of X / Wq / Wk / Wv in the exact SBUF tiling:
    halves input DMA bytes (16.8 -> 8.4 MB), gives 2KB-contiguous DMA
    segments, and removes all on-device weight/X casts from the vector
    engine (-34us DVE).  First matmul moves from ~38us to ~11us.
  * PE warm-up chain (32 fp16 junk matmuls) during the DMA head so the
    HAM clock gate reaches 8/8 before the first real projection.
  * Scores emitted qc-outer / half-inner: consecutive matmuls alternate
    row groups (tile_position (0,0)/(64,0)) so the two K=64 matmuls run
    CONCURRENTLY in the PE array (~2x scores throughput).
  * Constituency penalty fix as one masked tensor_mul per (half, kt)
    against a precomputed [128,128] mask (was 2 tensor_scalars).
  * V projection rebalanced: heads 0-7 in slot 0, heads 8-15 spread over
    slots 1-4 (slot 0 was 2x the PE work of other slots).
  * Split-chain AV epilogue: pair 7's AV contractions over kt 0..6 are
    issued before the last exps retire; only the kt=7 closers + transposes
    + ship remain after the final exp (tail 14us -> ~5us).
"""

import math
import sys

if "/opt/trn_rl_repo" not in sys.path:
    sys.path.insert(0, "/opt/trn_rl_repo")

import numpy as np

import concourse.bacc as bacc
import concourse.tile as tile
from concourse import mybir
from concourse.bass_utils import run_bass_kernel_spmd

F16 = mybir.dt.float16
F32 = mybir.dt.float32

B, S, H = 8, 1024, 1024
NH, HD = 16, 64
P = 128
SO = S // P   # 8 S-chunks
KO = H // P   # 8 contraction chunks
NP = NH // 2  # 8 head pairs
PEN = 0.5
FIX = float(math.exp(PEN))
SCALE = 1.0 / math.sqrt(HD)

_programs = {}


def _build_program(with_bv: bool):
    nc = bacc.Bacc("TRN2", target_bir_lowering=False, debug=False)

    # Host pre-laid-out fp16 inputs (see _in_maps for the exact packing).
    xt = nc.dram_tensor("xt", [P, KO, S], F16, kind="ExternalInput").ap()
    wq = nc.dram_tensor("wq", [P, NP, KO, P], F16, kind="ExternalInput").ap()
    wk = nc.dram_tensor("wk", [P, NP, KO, P], F16, kind="ExternalInput").ap()
    wv = nc.dram_tensor("wv", [P, KO, H], F16, kind="ExternalInput").ap()
    bq = nc.dram_tensor("bq", [P, KO], F32, kind="ExternalInput").ap()
    bk = nc.dram_tensor("bk", [P, KO], F32, kind="ExternalInput").ap()
    bv = nc.dram_tensor("bv", [H], F32, kind="ExternalInput").ap()
    out = nc.dram_tensor("out", [S, H], F32, kind="ExternalOutput").ap()

    Exp = mybir.ActivationFunctionType.Exp

    out_r = out.rearrange("(o p) n -> p o n", p=P)         # [128, SO, H]

    with tile.TileContext(nc) as tc:
        with (
            tc.tile_pool(name="persist", bufs=1) as persist,
            tc.tile_pool(name="probs", bufs=1) as probs,
            tc.tile_pool(name="score_ps", bufs=1, space="PSUM") as score_ps,
            tc.tile_pool(name="proj_ps", bufs=1, space="PSUM") as proj_ps,
            tc.tile_pool(name="work_ps", bufs=1, space="PSUM") as work_ps,
        ):
            # ---------------- persistent SBUF ----------------
            XT = persist.tile([P, KO, S], F16, name="XT")
            QT3 = persist.tile([P, 3, S], F16, name="QT3")
            KT3 = persist.tile([P, 3, S], F16, name="KT3")
            VA = persist.tile([P, SO, NH * (HD + 1)], F16, name="VA")
            wqh = persist.tile([P, NP, KO, P], F16, name="wqh")
            wkh = persist.tile([P, NP, KO, P], F16, name="wkh")
            wvh = persist.tile([P, KO, H], F16, name="wvh")
            nbias = persist.tile([P, 1], F32, name="nbias")
            bq_s = persist.tile([P, KO], F32, name="bq_s")
            bk_s = persist.tile([P, KO], F32, name="bk_s")
            pmask = persist.tile([P, P], F16, name="pmask")
            w16 = persist.tile([P, 512], F16, name="w16")

            # vector-engine setup (fast, unblocks warmup + exp + penalty)
            nc.vector.memset(w16[:], 1.0)
            nc.vector.memset(nbias[:], -PEN)
            nc.vector.memset(pmask[:], 1.0)
            nc.vector.memset(pmask[0:64, 0:64], FIX)
            nc.vector.memset(pmask[64:128, 64:128], FIX)
            # ones columns of V_aug only; V block copies fill the rest
            VA_v = VA[:].rearrange("p s (h c) -> p s h c", c=HD + 1)
            nc.vector.memset(VA_v[:, :, :, HD : HD + 1], 1.0)

            # ---------------- DMA: prioritized, fp16, contiguous ----------
            # 4 hw DMA queues for the head: sync (SP), scalar (Activation),
            # gpsimd, and vector (only its first-scores inputs; vector's
            # compute role starts later).  X chunks + first q/k slices
            # first; biases and Wv behind; late q/k slices last.
            nc.gpsimd.dma_start(out=bq_s[:], in_=bq)
            nc.gpsimd.dma_start(out=bk_s[:], in_=bk)
            nc.vector.tensor_scalar_mul(bq_s[:], bq_s[:], SCALE)
            nc.scalar.dma_start(out=wkh[:, 0, :, :], in_=wk[:, 0, :, :])
            nc.sync.dma_start(out=XT[:, 0:3, :], in_=xt[:, 0:3, :])
            nc.scalar.dma_start(out=XT[:, 3:6, :], in_=xt[:, 3:6, :])
            nc.gpsimd.dma_start(out=XT[:, 6:8, :], in_=xt[:, 6:8, :])
            nc.sync.dma_start(out=wqh[:, 0, :, :], in_=wq[:, 0, :, :])
            nc.gpsimd.dma_start(out=wqh[:, 1, :, :], in_=wq[:, 1, :, :])
            nc.gpsimd.dma_start(out=wkh[:, 1, :, :], in_=wk[:, 1, :, :])
            nc.scalar.dma_start(out=wvh[:, 0:4, :], in_=wv[:, 0:4, :])
            nc.sync.dma_start(out=wvh[:, 4:8, :], in_=wv[:, 4:8, :])
            for mo in range(2, NP):
                nc.gpsimd.dma_start(out=wqh[:, mo, :, :], in_=wq[:, mo, :, :])
                nc.gpsimd.dma_start(out=wkh[:, mo, :, :], in_=wk[:, mo, :, :])

            # ---------------- PE warm-up ----------------
            # ~48 junk matmuls (N=128, fp16) keep the PE busy from ~7us so
            # the HAM clock-gate is at 8/8 when the first projection lands.
            warm = work_ps.tile([P, 512], F32, name="warm", tag="work")
            for w in range(22):
                nc.tensor.matmul(
                    warm[:], w16[:, 0:P], w16[:],
                    start=(w == 0), stop=(w == 21),
                )

            # probs rings: 2 pairs in flight (scored / being consumed by AV)
            prT = [
                [
                    probs.tile([P, KO, S], F16, name=f"prT_{h}_{r}")
                    for r in range(2)
                ]
                for h in range(2)
            ]
            ctxt_sb = [
                probs.tile([80, 512], F16, name=f"ctxt_sb{r}")
                for r in range(4)
            ]
            tpx_sb = [
                probs.tile([P, 4, 80], F16, name=f"tpx_sb{r}")
                for r in range(4)
            ]
            for r in range(4):
                nc.vector.memset(ctxt_sb[r][HD : 80, :], 0.0)
            inv_sb = [
                probs.tile([P, 4], F32, name=f"inv{r}")
                for r in range(4)
            ]
            out_pair = [
                probs.tile([P, SO, P], F32, name=f"out_pair{r}")
                for r in range(2)
            ]

            # score psum: 3 rotating [128, S] tiles; a score pair only
            # WARs the exp from 2 steps back, so pairs issue concurrently
            pst = [
                score_ps.tile([P, S], F32, name=f"pst{h}", tag=f"pst{h}")
                for h in range(3)
            ]
            pst_ctr = [0]

            # ---------------- emission helpers ----------------
            def proj_burst(mo, which, sc, pool=None):
                """One Q or K projection burst: 8 chained MMs -> 1 psum bank,
                evacuated to the QT/KT ring with scale+bias fused."""
                wsb = wqh if which == "q" else wkh
                r = mo % 3
                pl = pool if pool is not None else proj_ps
                ps = pl.tile([P, 512], F32, name="ps",
                             tag="proj" if pl is proj_ps else "work")
                for kh in range(KO):
                    nc.tensor.matmul(
                        ps[:],
                        wsb[:, mo, kh, :],
                        XT[:, kh, sc * 512 : (sc + 1) * 512],
                        start=(kh == 0),
                        stop=(kh == KO - 1),
                    )
                if which == "q":
                    nc.vector.tensor_scalar(
                        QT3[:, r, sc * 512 : (sc + 1) * 512], ps[:],
                        SCALE, bq_s[:, mo : mo + 1],
                        mybir.AluOpType.mult, mybir.AluOpType.add,
                    )
                else:
                    nc.vector.tensor_scalar_add(
                        KT3[:, r, sc * 512 : (sc + 1) * 512], ps[:],
                        bk_s[:, mo : mo + 1],
                    )

            def v_burst(so, ncol):
                """V projection burst: 8 chained MMs -> work ring bank, then
                strided copy into V_aug (leaving the ones columns alone)."""
                ps = work_ps.tile([P, 512], F32, name="vps", tag="work")
                for kh in range(KO):
                    nc.tensor.matmul(
                        ps[:],
                        XT[:, kh, so * P : (so + 1) * P],
                        wvh[:, kh, ncol * 512 : (ncol + 1) * 512],
                        start=(kh == 0),
                        stop=(kh == KO - 1),
                    )
                va_v = VA[:, so, :].rearrange("p (h c) -> p h c", c=HD + 1)
                nc.vector.tensor_copy(
                    va_v[:, ncol * 8 : (ncol + 1) * 8, 0:HD],
                    ps[:].rearrange("p (h c) -> p h c", c=HD),
                )

            def score_qc(i, kt, qc):
                """One qc's pair of K=64 row-tiled score MMs (concurrent)."""
                r = i % 3
                base = pst_ctr[0]
                for half in range(2):
                    lo = half * 64
                    nc.tensor.matmul(
                        pst[(base + half) % 3][:, qc * 512 : (qc + 1) * 512],
                        KT3[lo : lo + 64, r, kt * P : (kt + 1) * P],
                        QT3[lo : lo + 64, r, qc * 512 : (qc + 1) * 512],
                        start=True,
                        stop=True,
                        tile_position=(lo, 0),
                    )
                if qc == 1:
                    pst_ctr[0] = base + 2

            def exp_kt(i, kt, ring):
                """Per-half exps (pst rotation makes the pairs concurrent)."""
                base = pst_ctr[0] - 2
                for half in range(2):
                    dst = prT[half][ring]
                    nc.scalar.activation(
                        dst[:, kt, :], pst[(base + half) % 3][:], Exp,
                        bias=nbias[:],
                    )
                    nc.vector.tensor_mul(
                        dst[:, kt, kt * P : (kt + 1) * P],
                        dst[:, kt, kt * P : (kt + 1) * P],
                        pmask[:],
                    )

            def av_open(i, half, qc, ring, pool, kt_hi):
                """Open an AV accumulation chain over kt 0..kt_hi-1."""
                ctx = pool.tile([P, 512], F32, name="ctx",
                                tag="proj" if pool is proj_ps else "work")
                return av_open_into(ctx, i, half, qc, ring, kt_hi)

            def av_open_into(ctx, i, half, qc, ring, kt_hi):
                h = 2 * i + half
                for kt in range(kt_hi):
                    nc.tensor.matmul(
                        ctx[0 : HD + 1, :],
                        VA[:, kt, h * (HD + 1) : (h + 1) * (HD + 1)],
                        prT[half][ring][:, kt, qc * 512 : (qc + 1) * 512],
                        start=(kt == 0),
                        stop=False,
                    )
                return ctx

            def av_close(ctx, i, half, qc, ring, slot4, kt_lo, eng_scalar=False):
                """Close an AV chain (kt_lo..7) and evacuate to SBUF."""
                h = 2 * i + half
                for kt in range(kt_lo, KO):
                    nc.tensor.matmul(
                        ctx[0 : HD + 1, :],
                        VA[:, kt, h * (HD + 1) : (h + 1) * (HD + 1)],
                        prT[half][ring][:, kt, qc * 512 : (qc + 1) * 512],
                        start=False,
                        stop=(kt == KO - 1),
                    )
                if eng_scalar:
                    nc.scalar.copy(ctxt_sb[slot4][0 : HD + 1, :], ctx[0 : HD + 1, :])
                else:
                    nc.vector.tensor_copy(
                        ctxt_sb[slot4][0 : HD + 1, :], ctx[0 : HD + 1, :]
                    )

            def av_burst(i, half, qc, ring, slot4):
                """Full AV for (pair i, head-half, q-chunk qc)."""
                ctx = av_open(i, half, qc, ring, work_ps, KO - 1)
                av_close(ctx, i, half, qc, ring, slot4, KO - 1)

            def tp_burst(half, qc, slot4, opr, pool=None, eng=None):
                """X-bar DMA-transpose ctx^T back to [q, d] + normalize."""
                tpx = tpx_sb[slot4]
                (eng or nc.sync).dma_start(out=tpx[:], in_=ctxt_sb[slot4][:],
                                           transpose=True)
                nc.vector.reciprocal(inv_sb[slot4][:], tpx[:, :, HD])
                for c4 in range(4):
                    so = qc * 4 + c4
                    nc.vector.tensor_scalar_mul(
                        out_pair[opr][:, so, half * HD : (half + 1) * HD],
                        tpx[:, c4, 0:HD],
                        inv_sb[slot4][:, c4 : c4 + 1],
                    )

            def ship_pair(i, opr):
                nc.sync.dma_start(
                    out=out_r[:, :, i * P : (i + 1) * P], in_=out_pair[opr][:]
                )

            # ---------------- bootstrap: proj(0) ----------------
            proj_burst(0, "q", 0)
            proj_burst(0, "k", 0, pool=work_ps)
            proj_burst(0, "q", 1)
            proj_burst(0, "k", 1, pool=work_ps)

            # ---------------- pair slots ----------------
            # slot i: scores(i) + exp(i) | proj(i+1) | V bursts (slot 0 +
            # spread) | i>=1: AV(i-1) + tp(i-1) + ship(i-1)
            av_seq = [(0, 0), (0, 1), (1, 0), (1, 1)]
            for i in range(NP):
                ring = i % 2
                pring = (i - 1) % 2
                opr = (i - 1) % 2

                for kt in range(KO):
                    score_qc(i, kt, 0)

                    # interleaved burst between the two score pairs
                    if i + 1 < NP:
                        if kt == 0:
                            proj_burst(i + 1, "q", 0)
                        elif kt == 2:
                            proj_burst(i + 1, "q", 1)
                        elif kt == 4:
                            proj_burst(i + 1, "k", 0)
                        elif kt == 6:
                            proj_burst(i + 1, "k", 1)
                    if i == 0:
                        if with_bv:
                            if kt < 4:
                                v_burst(2 * kt, 0)
                                v_burst(2 * kt, 1)
                            else:
                                v_burst(2 * (kt - 4) + 1, 0)
                                v_burst(2 * (kt - 4) + 1, 1)
                        else:
                            # heads 0-7 here; heads 8-15 spread over slots 1-4
                            v_burst(kt, 0)
                    elif kt % 2 == 0:
                        half, qc = av_seq[kt // 2]
                        av_burst(i - 1, half, qc, pring, kt // 2)

                    score_qc(i, kt, 1)

                    if i >= 1:
                        if kt % 2 == 1 and kt >= 3:
                            half, qc = av_seq[(kt - 3) // 2]
                            tp_burst(half, qc, (kt - 3) // 2, opr)
                        if not with_bv and 1 <= i <= 4 and kt in (1, 5):
                            v_burst(2 * (i - 1) + (0 if kt == 1 else 1), 1)

                    exp_kt(i, kt, ring)

                if i == 0 and with_bv:
                    # out += bv exactly (softmax rows sum to 1): broadcast bv
                    # across partitions via PE, add into V_aug.
                    ones1 = persist.tile([1, P], F16, name="ones1")
                    nc.vector.memset(ones1[:], 1.0)
                    bv1 = persist.tile([1, H], F16, name="bv1")
                    bv1_32 = persist.tile([1, H], F32, name="bv1_32")
                    nc.sync.dma_start(out=bv1_32[:], in_=bv[None, :])
                    nc.vector.tensor_copy(bv1[:], bv1_32[:])
                    bvb = persist.tile([P, NH * (HD + 1)], F16, name="bvb")
                    nc.vector.memset(bvb[:], 0.0)
                    bvb_v = bvb.rearrange("p (h c) -> p h c", c=HD + 1)
                    for ncol in range(2):
                        psb = proj_ps.tile([P, 512], F32, name="psb", tag="proj")
                        nc.tensor.matmul(
                            psb[:], ones1[:], bv1[:, ncol * 512 : (ncol + 1) * 512],
                            start=True, stop=True,
                        )
                        nc.vector.tensor_copy(
                            bvb_v[:, ncol * 8 : (ncol + 1) * 8, 0:HD],
                            psb[:].rearrange("p (h c) -> p h c", c=HD),
                        )
                    for so in range(SO):
                        nc.vector.tensor_add(VA[:, so, :], VA[:, so, :], bvb[:])

                if i >= 1:
                    # last tp of pair i-1 spills here; then ship the pair
                    tp_burst(1, 1, 3, opr)
                    ship_pair(i - 1, opr)

            # ---------------- epilogue: AV(7) + tp(7), split-chain --------
            # Open all four AV chains over kt 0..6 (their exps are done while
            # kt=7's exps still run); only the kt=7 closers + transposes +
            # ship are exposed after the final exp.
            ring = (NP - 1) % 2
            opr = (NP - 1) % 2
            # pst tiles for the two tags NOT used by kt7 are free once
            # their last exps (kt6-h1 / kt7-h0) retire -- reuse their banks
            # for two of the split AV chains.
            b7 = (pst_ctr[0] - 2) % 3          # kt7 h0 tag
            free_tags = [t for t in range(3) if t not in (b7, (b7 + 1) % 3)]
            ps_e = score_ps.tile([P, S], F32, name="ps_e", tag=f"pst{free_tags[0]}")
            ps_f = score_ps.tile([P, S], F32, name="ps_f", tag=f"pst{b7}")
            cA0 = av_open(NP - 1, 0, 0, ring, proj_ps, KO - 1)
            cA1 = av_open(NP - 1, 0, 1, ring, work_ps, KO - 1)
            cA2 = av_open_into(ps_e[:, 0:512], NP - 1, 1, 0, ring, KO - 1)
            cA3 = av_open_into(ps_f[:, 0:512], NP - 1, 1, 1, ring, KO - 1)
            av_close(cA0, NP - 1, 0, 0, ring, 0, KO - 1)
            tp_burst(0, 0, 0, opr, eng=nc.sync)
            av_close(cA1, NP - 1, 0, 1, ring, 1, KO - 1)
            tp_burst(0, 1, 1, opr, eng=nc.scalar)
            av_close(cA2, NP - 1, 1, 0, ring, 2, KO - 1)
            tp_burst(1, 0, 2, opr, eng=nc.sync)
            av_close(cA3, NP - 1, 1, 1, ring, 3, KO - 1)
            tp_burst(1, 1, 3, opr, eng=nc.scalar)
            ship_pair(NP - 1, opr)

    nc.compile()
    return nc


def _get_program(with_bv: bool):
    key = with_bv
    if key not in _programs:
        _programs[key] = _build_program(with_bv)
    return _programs[key]


def _in_maps(hidden_states, Wq, bq, Wk, bk, Wv, bv):
    # fp16 pre-layout matching the SBUF tiling (see _build_program):
    #   xt[kp, ko, s]     = X[s, ko*128+kp]
    #   wq[kp, mo, ko, c] = Wq[ko*128+kp, mo*128+c]   (wk likewise)
    #   wv[kp, ko, n]     = Wv[ko*128+kp, n]
    wq16 = np.ascontiguousarray(
        np.asarray(Wq, np.float32).astype(np.float16)
        .reshape(KO, P, NP, P).transpose(1, 2, 0, 3)
    )
    wk16 = np.ascontiguousarray(
        np.asarray(Wk, np.float32).astype(np.float16)
        .reshape(KO, P, NP, P).transpose(1, 2, 0, 3)
    )
    wv16 = np.ascontiguousarray(
        np.asarray(Wv, np.float32).astype(np.float16)
        .reshape(KO, P, H).transpose(1, 0, 2)
    )
    bq = np.ascontiguousarray(
        np.asarray(bq, np.float32).reshape(KO, P).T)
    bk = np.ascontiguousarray(
        np.asarray(bk, np.float32).reshape(KO, P).T)
    bv = np.ascontiguousarray(bv, np.float32)
    maps = []
    for b in range(B):
        xt16 = np.ascontiguousarray(
            np.asarray(hidden_states[b], np.float32).astype(np.float16)
            .T.reshape(KO, P, S).transpose(1, 0, 2)
        )
        maps.append({
            "xt": xt16,
            "wq": wq16, "wk": wk16, "wv": wv16,
            "bq": bq, "bk": bk, "bv": bv,
        })
    return maps


def kernel(hidden_states, Wq, bq, Wk, bk, Wv, bv):
    hidden_states = np.ascontiguousarray(hidden_states, dtype=np.float32)
    with_bv = bool(np.any(np.asarray(bv) != 0))
    nc = _get_program(with_bv)
    in_maps = _in_maps(hidden_states, Wq, bq, Wk, bk, Wv, bv)
    last_err = None
    for _attempt in range(3):
        try:
            res = run_bass_kernel_spmd(nc, in_maps, list(range(B)))
            return np.stack([res.results[b]["out"] for b in range(B)], axis=0)
        except Exception as e:  # transient NRT device errors recover on retry
            last_err = e
            import time
            time.sleep(3)
    raise last_err


# revision 12
# speedup vs baseline: 1.0816x; 1.0665x over previous
"""ConstituencyAwareAttention Trainium2 kernel (v3).

Strategy: pure data parallelism -- B=8 batch elements across 8 NeuronCores,
one full attention problem per core (S=1024, H=1024, nh=16, hd=64).

v3 restructure (vs v2 baseline at ~283us):
  * Host-side fp16 pre-layout of X / Wq / Wk / Wv in the exact SBUF tiling:
    halves input DMA bytes (16.8 -> 8.4 MB), gives 2KB-contiguous DMA
    segments, and removes all on-device weight/X casts from the vector
    engine (-34us DVE).  First matmul moves from ~38us to ~11us.
  * PE warm-up chain (32 fp16 junk matmuls) during the DMA head so the
    HAM clock gate reaches 8/8 before the first real projection.
  * Scores emitted qc-outer / half-inner: consecutive matmuls alternate
    row groups (tile_position (0,0)/(64,0)) so the two K=64 matmuls run
    CONCURRENTLY in the PE array (~2x scores throughput).
  * Constituency penalty fix as one masked tensor_mul per (half, kt)
    against a precomputed [128,128] mask (was 2 tensor_scalars).
  * V projection rebalanced: heads 0-7 in slot 0, heads 8-15 spread over
    slots 1-4 (slot 0 was 2x the PE work of other slots).
  * Split-chain AV epilogue: pair 7's AV contractions over kt 0..6 are
    issued before the last exps retire; only the kt=7 closers + transposes
    + ship remain after the final exp (tail 14us -> ~5us).
"""

import math
import sys

if "/opt/trn_rl_repo" not in sys.path:
    sys.path.insert(0, "/opt/trn_rl_repo")

import numpy as np

import concourse.bacc as bacc
import concourse.tile as tile
from concourse import mybir
from concourse.bass_utils import run_bass_kernel_spmd

F16 = mybir.dt.float16
F32 = mybir.dt.float32

B, S, H = 8, 1024, 1024
NH, HD = 16, 64
P = 128
SO = S // P   # 8 S-chunks
KO = H // P   # 8 contraction chunks
NP = NH // 2  # 8 head pairs
PEN = 0.5
FIX = float(math.exp(PEN))
SCALE = 1.0 / math.sqrt(HD)

_programs = {}


def _build_program(with_bv: bool):
    nc = bacc.Bacc("TRN2", target_bir_lowering=False, debug=False)

    # Host pre-laid-out fp16 inputs (see _in_maps for the exact packing).
    xt = nc.dram_tensor("xt", [P, KO, S], F16, kind="ExternalInput").ap()
    wq = nc.dram_tensor("wq", [P, NP, KO, P], F16, kind="ExternalInput").ap()
    wk = nc.dram_tensor("wk", [P, NP, KO, P], F16, kind="ExternalInput").ap()
    wv = nc.dram_tensor("wv", [P, KO, H], F16, kind="ExternalInput").ap()
    bq = nc.dram_tensor("bq", [P, KO], F32, kind="ExternalInput").ap()
    bk = nc.dram_tensor("bk", [P, KO], F32, kind="ExternalInput").ap()
    bv = nc.dram_tensor("bv", [H], F32, kind="ExternalInput").ap()
    out = nc.dram_tensor("out", [S, H], F32, kind="ExternalOutput").ap()

    Exp = mybir.ActivationFunctionType.Exp

    out_r = out.rearrange("(o p) n -> p o n", p=P)         # [128, SO, H]

    with tile.TileContext(nc) as tc:
        with (
            tc.tile_pool(name="persist", bufs=1) as persist,
            tc.tile_pool(name="probs", bufs=1) as probs,
            tc.tile_pool(name="score_ps", bufs=1, space="PSUM") as score_ps,
            tc.tile_pool(name="proj_ps", bufs=1, space="PSUM") as proj_ps,
            tc.tile_pool(name="work_ps", bufs=1, space="PSUM") as work_ps,
        ):
            # ---------------- persistent SBUF ----------------
            XT = persist.tile([P, KO, S], F16, name="XT")
            QT3 = persist.tile([P, 3, S], F16, name="QT3")
            KT3 = persist.tile([P, 3, S], F16, name="KT3")
            VA = persist.tile([P, SO, NH * (HD + 1)], F16, name="VA")
            wqh = persist.tile([P, NP, KO, P], F16, name="wqh")
            wkh = persist.tile([P, NP, KO, P], F16, name="wkh")
            wvh = persist.tile([P, KO, H], F16, name="wvh")
            nbias = persist.tile([P, 1], F32, name="nbias")
            bq_s = persist.tile([P, KO], F32, name="bq_s")
            bk_s = persist.tile([P, KO], F32, name="bk_s")
            pmask = persist.tile([P, P], F16, name="pmask")
            w16 = persist.tile([P, 512], F16, name="w16")

            # vector-engine setup (fast, unblocks warmup + exp + penalty)
            nc.vector.memset(w16[:], 1.0)
            nc.vector.memset(nbias[:], -PEN)
            nc.vector.memset(pmask[:], 1.0)
            nc.vector.memset(pmask[0:64, 0:64], FIX)
            nc.vector.memset(pmask[64:128, 64:128], FIX)
            # ones columns of V_aug only; V block copies fill the rest
            VA_v = VA[:].rearrange("p s (h c) -> p s h c", c=HD + 1)
            nc.vector.memset(VA_v[:, :, :, HD : HD + 1], 1.0)

            # ---------------- DMA: prioritized, fp16, contiguous ----------
            # 4 hw DMA queues for the head: sync (SP), scalar (Activation),
            # gpsimd, and vector (only its first-scores inputs; vector's
            # compute role starts later).  X chunks + first q/k slices
            # first; biases and Wv behind; late q/k slices last.
            nc.gpsimd.dma_start(out=bq_s[:], in_=bq)
            nc.gpsimd.dma_start(out=bk_s[:], in_=bk)
            nc.vector.tensor_scalar_mul(bq_s[:], bq_s[:], SCALE)
            nc.scalar.dma_start(out=wkh[:, 0, :, :], in_=wk[:, 0, :, :])
            nc.sync.dma_start(out=XT[:, 0:3, :], in_=xt[:, 0:3, :])
            nc.scalar.dma_start(out=XT[:, 3:6, :], in_=xt[:, 3:6, :])
            nc.gpsimd.dma_start(out=XT[:, 6:8, :], in_=xt[:, 6:8, :])
            nc.sync.dma_start(out=wqh[:, 0, :, :], in_=wq[:, 0, :, :])
            nc.gpsimd.dma_start(out=wqh[:, 1, :, :], in_=wq[:, 1, :, :])
            nc.gpsimd.dma_start(out=wkh[:, 1, :, :], in_=wk[:, 1, :, :])
            nc.scalar.dma_start(out=wvh[:, 0:4, :], in_=wv[:, 0:4, :])
            nc.sync.dma_start(out=wvh[:, 4:8, :], in_=wv[:, 4:8, :])
            for mo in range(2, NP):
                nc.gpsimd.dma_start(out=wqh[:, mo, :, :], in_=wq[:, mo, :, :])
                nc.gpsimd.dma_start(out=wkh[:, mo, :, :], in_=wk[:, mo, :, :])

            # ---------------- PE warm-up ----------------
            # ~48 junk matmuls (N=128, fp16) keep the PE busy from ~7us so
            # the HAM clock-gate is at 8/8 when the first projection lands.
            warm = work_ps.tile([P, 512], F32, name="warm", tag="work")
            for w in range(22):
                nc.tensor.matmul(
                    warm[:], w16[:, 0:P], w16[:],
                    start=(w == 0), stop=(w == 21),
                )

            # probs rings: 2 pairs in flight (scored / being consumed by AV)
            prT = [
                [
                    probs.tile([P, KO, S], F16, name=f"prT_{h}_{r}")
                    for r in range(2)
                ]
                for h in range(2)
            ]
            ctxt_sb = [
                probs.tile([80, 512], F16, name=f"ctxt_sb{r}")
                for r in range(4)
            ]
            tpx_sb = [
                probs.tile([P, 4, 80], F16, name=f"tpx_sb{r}")
                for r in range(4)
            ]
            for r in range(4):
                nc.vector.memset(ctxt_sb[r][HD : 80, :], 0.0)
            inv_sb = [
                probs.tile([P, 4], F32, name=f"inv{r}")
                for r in range(4)
            ]
            out_pair = [
                probs.tile([P, SO, P], F32, name=f"out_pair{r}")
                for r in range(2)
            ]

            # score psum: 3 rotating [128, S] tiles; a score pair only
            # WARs the exp from 2 steps back, so pairs issue concurrently
            pst = [
                score_ps.tile([P, S], F32, name=f"pst{h}", tag=f"pst{h}")
                for h in range(3)
            ]
            pst_ctr = [0]

            # ---------------- emission helpers ----------------
            def proj_burst(mo, which, sc, pool=None):
                """One Q or K projection burst: 8 chained MMs -> 1 psum bank,
                evacuated to the QT/KT ring with scale+bias fused."""
                wsb = wqh if which == "q" else wkh
                r = mo % 3
                pl = pool if pool is not None else proj_ps
                ps = pl.tile([P, 512], F32, name="ps",
                             tag="proj" if pl is proj_ps else "work")
                for kh in range(KO):
                    nc.tensor.matmul(
                        ps[:],
                        wsb[:, mo, kh, :],
                        XT[:, kh, sc * 512 : (sc + 1) * 512],
                        start=(kh == 0),
                        stop=(kh == KO - 1),
                    )
                if which == "q":
                    nc.vector.tensor_scalar(
                        QT3[:, r, sc * 512 : (sc + 1) * 512], ps[:],
                        SCALE, bq_s[:, mo : mo + 1],
                        mybir.AluOpType.mult, mybir.AluOpType.add,
                    )
                else:
                    nc.vector.tensor_scalar_add(
                        KT3[:, r, sc * 512 : (sc + 1) * 512], ps[:],
                        bk_s[:, mo : mo + 1],
                    )

            def v_burst(so, ncol):
                """V projection burst: 8 chained MMs -> work ring bank, then
                strided copy into V_aug (leaving the ones columns alone)."""
                ps = work_ps.tile([P, 512], F32, name="vps", tag="work")
                for kh in range(KO):
                    nc.tensor.matmul(
                        ps[:],
                        XT[:, kh, so * P : (so + 1) * P],
                        wvh[:, kh, ncol * 512 : (ncol + 1) * 512],
                        start=(kh == 0),
                        stop=(kh == KO - 1),
                    )
                va_v = VA[:, so, :].rearrange("p (h c) -> p h c", c=HD + 1)
                nc.vector.tensor_copy(
                    va_v[:, ncol * 8 : (ncol + 1) * 8, 0:HD],
                    ps[:].rearrange("p (h c) -> p h c", c=HD),
                )

            def score_qc(i, kt, qc):
                """One qc's pair of K=64 row-tiled score MMs (concurrent)."""
                r = i % 3
                base = pst_ctr[0]
                for half in range(2):
                    lo = half * 64
                    nc.tensor.matmul(
                        pst[(base + half) % 3][:, qc * 512 : (qc + 1) * 512],
                        KT3[lo : lo + 64, r, kt * P : (kt + 1) * P],
                        QT3[lo : lo + 64, r, qc * 512 : (qc + 1) * 512],
                        start=True,
                        stop=True,
                        tile_position=(lo, 0),
                    )
                if qc == 1:
                    pst_ctr[0] = base + 2

            def exp_kt(i, kt, ring):
                """Per-half exps (pst rotation makes the pairs concurrent)."""
                base = pst_ctr[0] - 2
                for half in range(2):
                    dst = prT[half][ring]
                    nc.scalar.activation(
                        dst[:, kt, :], pst[(base + half) % 3][:], Exp,
                        bias=nbias[:],
                    )
                    nc.vector.tensor_mul(
                        dst[:, kt, kt * P : (kt + 1) * P],
                        dst[:, kt, kt * P : (kt + 1) * P],
                        pmask[:],
                    )

            def av_open(i, half, qc, ring, pool, kt_hi):
                """Open an AV accumulation chain over kt 0..kt_hi-1."""
                ctx = pool.tile([P, 512], F32, name="ctx",
                                tag="proj" if pool is proj_ps else "work")
                return av_open_into(ctx, i, half, qc, ring, kt_hi)

            def av_open_into(ctx, i, half, qc, ring, kt_hi):
                h = 2 * i + half
                for kt in range(kt_hi):
                    nc.tensor.matmul(
                        ctx[0 : HD + 1, :],
                        VA[:, kt, h * (HD + 1) : (h + 1) * (HD + 1)],
                        prT[half][ring][:, kt, qc * 512 : (qc + 1) * 512],
                        start=(kt == 0),
                        stop=False,
                    )
                return ctx

            def av_close(ctx, i, half, qc, ring, slot4, kt_lo, eng_scalar=False):
                """Close an AV chain (kt_lo..7) and evacuate to SBUF."""
                h = 2 * i + half
                for kt in range(kt_lo, KO):
                    nc.tensor.matmul(
                        ctx[0 : HD + 1, :],
                        VA[:, kt, h * (HD + 1) : (h + 1) * (HD + 1)],
                        prT[half][ring][:, kt, qc * 512 : (qc + 1) * 512],
                        start=False,
                        stop=(kt == KO - 1),
                    )
                if eng_scalar:
                    nc.scalar.copy(ctxt_sb[slot4][0 : HD + 1, :], ctx[0 : HD + 1, :])
                else:
                    nc.vector.tensor_copy(
                        ctxt_sb[slot4][0 : HD + 1, :], ctx[0 : HD + 1, :]
                    )

            def av_burst(i, half, qc, ring, slot4):
                """Full AV for (pair i, head-half, q-chunk qc)."""
                ctx = av_open(i, half, qc, ring, work_ps, KO - 1)
                av_close(ctx, i, half, qc, ring, slot4, KO - 1)

            def tp_burst(half, qc, slot4, opr, pool=None, eng=None):
                """X-bar DMA-transpose ctx^T back to [q, d] + normalize."""
                tpx = tpx_sb[slot4]
                (eng or nc.sync).dma_start(out=tpx[:], in_=ctxt_sb[slot4][:],
                                           transpose=True)
                nc.vector.reciprocal(inv_sb[slot4][:], tpx[:, :, HD])
                for c4 in range(4):
                    so = qc * 4 + c4
                    nc.vector.tensor_scalar_mul(
                        out_pair[opr][:, so, half * HD : (half + 1) * HD],
                        tpx[:, c4, 0:HD],
                        inv_sb[slot4][:, c4 : c4 + 1],
                    )

            def ship_pair(i, opr):
                nc.sync.dma_start(
                    out=out_r[:, :, i * P : (i + 1) * P], in_=out_pair[opr][:]
                )

            # ---------------- bootstrap: proj(0) ----------------
            proj_burst(0, "q", 0)
            proj_burst(0, "k", 0, pool=work_ps)
            proj_burst(0, "q", 1)
            proj_burst(0, "k", 1, pool=work_ps)

            # ---------------- pair slots ----------------
            # slot i: scores(i) + exp(i) | proj(i+1) | V bursts (slot 0 +
            # spread) | i>=1: AV(i-1) + tp(i-1) + ship(i-1)
            av_seq = [(0, 0), (0, 1), (1, 0), (1, 1)]
            for i in range(NP):
                ring = i % 2
                pring = (i - 1) % 2
                opr = (i - 1) % 2

                for kt in range(KO):
                    # all four score MMs contiguous: each qc's half-pair runs
                    # concurrently (pst rotation), and only one K64<->K128
                    # transition pair per kt (transitions cost ~100ns extra)
                    score_qc(i, kt, 0)
                    score_qc(i, kt, 1)
                    exp_kt(i, kt, ring)

                    if i + 1 < NP:
                        if kt == 0:
                            proj_burst(i + 1, "q", 0)
                        elif kt == 2:
                            proj_burst(i + 1, "q", 1)
                        elif kt == 4:
                            proj_burst(i + 1, "k", 0)
                        elif kt == 6:
                            proj_burst(i + 1, "k", 1)
                    if i == 0:
                        if with_bv:
                            if kt < 4:
                                v_burst(2 * kt, 0)
                                v_burst(2 * kt, 1)
                            else:
                                v_burst(2 * (kt - 4) + 1, 0)
                                v_burst(2 * (kt - 4) + 1, 1)
                        else:
                            # heads 0-7 here; heads 8-15 spread over slots 1-4
                            v_burst(kt, 0)
                    else:
                        if kt % 2 == 0:
                            half, qc = av_seq[kt // 2]
                            av_burst(i - 1, half, qc, pring, kt // 2)
                        elif kt >= 3:
                            half, qc = av_seq[(kt - 3) // 2]
                            tp_burst(half, qc, (kt - 3) // 2, opr)
                        if not with_bv and 1 <= i <= 4 and kt in (1, 5):
                            v_burst(2 * (i - 1) + (0 if kt == 1 else 1), 1)

                if i == 0 and with_bv:
                    # out += bv exactly (softmax rows sum to 1): broadcast bv
                    # across partitions via PE, add into V_aug.
                    ones1 = persist.tile([1, P], F16, name="ones1")
                    nc.vector.memset(ones1[:], 1.0)
                    bv1 = persist.tile([1, H], F16, name="bv1")
                    bv1_32 = persist.tile([1, H], F32, name="bv1_32")
                    nc.sync.dma_start(out=bv1_32[:], in_=bv[None, :])
                    nc.vector.tensor_copy(bv1[:], bv1_32[:])
                    bvb = persist.tile([P, NH * (HD + 1)], F16, name="bvb")
                    nc.vector.memset(bvb[:], 0.0)
                    bvb_v = bvb.rearrange("p (h c) -> p h c", c=HD + 1)
                    for ncol in range(2):
                        psb = proj_ps.tile([P, 512], F32, name="psb", tag="proj")
                        nc.tensor.matmul(
                            psb[:], ones1[:], bv1[:, ncol * 512 : (ncol + 1) * 512],
                            start=True, stop=True,
                        )
                        nc.vector.tensor_copy(
                            bvb_v[:, ncol * 8 : (ncol + 1) * 8, 0:HD],
                            psb[:].rearrange("p (h c) -> p h c", c=HD),
                        )
                    for so in range(SO):
                        nc.vector.tensor_add(VA[:, so, :], VA[:, so, :], bvb[:])

                if i >= 1:
                    # last tp of pair i-1 spills here; then ship the pair
                    tp_burst(1, 1, 3, opr)
                    ship_pair(i - 1, opr)

            # ---------------- epilogue: AV(7) + tp(7), split-chain --------
            # Open all four AV chains over kt 0..6 (their exps are done while
            # kt=7's exps still run); only the kt=7 closers + transposes +
            # ship are exposed after the final exp.
            ring = (NP - 1) % 2
            opr = (NP - 1) % 2
            # pst tiles for the two tags NOT used by kt7 are free once
            # their last exps (kt6-h1 / kt7-h0) retire -- reuse their banks
            # for two of the split AV chains.
            b7 = (pst_ctr[0] - 2) % 3          # kt7 h0 tag
            free_tags = [t for t in range(3) if t not in (b7, (b7 + 1) % 3)]
            ps_e = score_ps.tile([P, S], F32, name="ps_e", tag=f"pst{free_tags[0]}")
            ps_f = score_ps.tile([P, S], F32, name="ps_f", tag=f"pst{b7}")
            cA0 = av_open(NP - 1, 0, 0, ring, proj_ps, KO - 1)
            cA1 = av_open(NP - 1, 0, 1, ring, work_ps, KO - 1)
            cA2 = av_open_into(ps_e[:, 0:512], NP - 1, 1, 0, ring, KO - 1)
            cA3 = av_open_into(ps_f[:, 0:512], NP - 1, 1, 1, ring, KO - 1)
            av_close(cA0, NP - 1, 0, 0, ring, 0, KO - 1)
            tp_burst(0, 0, 0, opr, eng=nc.sync)
            av_close(cA1, NP - 1, 0, 1, ring, 1, KO - 1)
            tp_burst(0, 1, 1, opr, eng=nc.scalar)
            av_close(cA2, NP - 1, 1, 0, ring, 2, KO - 1)
            tp_burst(1, 0, 2, opr, eng=nc.sync)
            av_close(cA3, NP - 1, 1, 1, ring, 3, KO - 1)
            tp_burst(1, 1, 3, opr, eng=nc.scalar)
            ship_pair(NP - 1, opr)

    nc.compile()
    return nc


def _get_program(with_bv: bool):
    key = with_bv
    if key not in _programs:
        _programs[key] = _build_program(with_bv)
    return _programs[key]


def _in_maps(hidden_states, Wq, bq, Wk, bk, Wv, bv):
    # fp16 pre-layout matching the SBUF tiling (see _build_program):
    #   xt[kp, ko, s]     = X[s, ko*128+kp]
    #   wq[kp, mo, ko, c] = Wq[ko*128+kp, mo*128+c]   (wk likewise)
    #   wv[kp, ko, n]     = Wv[ko*128+kp, n]
    wq16 = np.ascontiguousarray(
        np.asarray(Wq, np.float32).astype(np.float16)
        .reshape(KO, P, NP, P).transpose(1, 2, 0, 3)
    )
    wk16 = np.ascontiguousarray(
        np.asarray(Wk, np.float32).astype(np.float16)
        .reshape(KO, P, NP, P).transpose(1, 2, 0, 3)
    )
    wv16 = np.ascontiguousarray(
        np.asarray(Wv, np.float32).astype(np.float16)
        .reshape(KO, P, H).transpose(1, 0, 2)
    )
    bq = np.ascontiguousarray(
        np.asarray(bq, np.float32).reshape(KO, P).T)
    bk = np.ascontiguousarray(
        np.asarray(bk, np.float32).reshape(KO, P).T)
    bv = np.ascontiguousarray(bv, np.float32)
    maps = []
    for b in range(B):
        xt16 = np.ascontiguousarray(
            np.asarray(hidden_states[b], np.float32).astype(np.float16)
            .T.reshape(KO, P, S).transpose(1, 0, 2)
        )
        maps.append({
            "xt": xt16,
            "wq": wq16, "wk": wk16, "wv": wv16,
            "bq": bq, "bk": bk, "bv": bv,
        })
    return maps


def kernel(hidden_states, Wq, bq, Wk, bk, Wv, bv):
    hidden_states = np.ascontiguousarray(hidden_states, dtype=np.float32)
    with_bv = bool(np.any(np.asarray(bv) != 0))
    nc = _get_program(with_bv)
    in_maps = _in_maps(hidden_states, Wq, bq, Wk, bk, Wv, bv)
    last_err = None
    for _attempt in range(3):
        try:
            res = run_bass_kernel_spmd(nc, in_maps, list(range(B)))
            return np.stack([res.results[b]["out"] for b in range(B)], axis=0)
        except Exception as e:  # transient NRT device errors recover on retry
            last_err = e
            import time
            time.sleep(3)
    raise last_err


# revision 14
# speedup vs baseline: 1.1388x; 1.0529x over previous
"""ConstituencyAwareAttention Trainium2 kernel (v3).

Strategy: pure data parallelism -- B=8 batch elements across 8 NeuronCores,
one full attention problem per core (S=1024, H=1024, nh=16, hd=64).

v3 restructure (vs v2 baseline at ~283us):
  * Host-side fp16 pre-layout of X / Wq / Wk / Wv in the exact SBUF tiling:
    halves input DMA bytes (16.8 -> 8.4 MB), gives 2KB-contiguous DMA
    segments, and removes all on-device weight/X casts from the vector
    engine (-34us DVE).  First matmul moves from ~38us to ~11us.
  * PE warm-up chain (32 fp16 junk matmuls) during the DMA head so the
    HAM clock gate reaches 8/8 before the first real projection.
  * Scores emitted qc-outer / half-inner: consecutive matmuls alternate
    row groups (tile_position (0,0)/(64,0)) so the two K=64 matmuls run
    CONCURRENTLY in the PE array (~2x scores throughput).
  * Constituency penalty fix as one masked tensor_mul per (half, kt)
    against a precomputed [128,128] mask (was 2 tensor_scalars).
  * V projection rebalanced: heads 0-7 in slot 0, heads 8-15 spread over
    slots 1-4 (slot 0 was 2x the PE work of other slots).
  * Split-chain AV epilogue: pair 7's AV contractions over kt 0..6 are
    issued before the last exps retire; only the kt=7 closers + transposes
    + ship remain after the final exp (tail 14us -> ~5us).
"""

import math
import sys

if "/opt/trn_rl_repo" not in sys.path:
    sys.path.insert(0, "/opt/trn_rl_repo")

import numpy as np

import concourse.bacc as bacc
import concourse.tile as tile
from concourse import mybir
from concourse.bass_utils import run_bass_kernel_spmd

F16 = mybir.dt.float16
F32 = mybir.dt.float32

B, S, H = 8, 1024, 1024
NH, HD = 16, 64
P = 128
SO = S // P   # 8 S-chunks
KO = H // P   # 8 contraction chunks
NP = NH // 2  # 8 head pairs
PEN = 0.5
FIX = float(math.exp(PEN))
SCALE = 1.0 / math.sqrt(HD)

_programs = {}


def _build_program(with_bv: bool):
    nc = bacc.Bacc("TRN2", target_bir_lowering=False, debug=False)

    # Host pre-laid-out fp16 inputs (see _in_maps for the exact packing).
    xt = nc.dram_tensor("xt", [P, KO, S], F16, kind="ExternalInput").ap()
    wq = nc.dram_tensor("wq", [P, NP, KO, P], F16, kind="ExternalInput").ap()
    wk = nc.dram_tensor("wk", [P, NP, KO, P], F16, kind="ExternalInput").ap()
    wv = nc.dram_tensor("wv", [P, KO, H], F16, kind="ExternalInput").ap()
    bq = nc.dram_tensor("bq", [P, KO], F32, kind="ExternalInput").ap()
    bk = nc.dram_tensor("bk", [P, KO], F32, kind="ExternalInput").ap()
    bv = nc.dram_tensor("bv", [H], F32, kind="ExternalInput").ap()
    out = nc.dram_tensor("out", [S, H], F16, kind="ExternalOutput").ap()

    Exp = mybir.ActivationFunctionType.Exp

    out_r = out.rearrange("(o p) n -> p o n", p=P)         # [128, SO, H]

    with tile.TileContext(nc) as tc:
        with (
            tc.tile_pool(name="persist", bufs=1) as persist,
            tc.tile_pool(name="probs", bufs=1) as probs,
            tc.tile_pool(name="score_ps", bufs=1, space="PSUM") as score_ps,
            tc.tile_pool(name="proj_ps", bufs=1, space="PSUM") as proj_ps,
            tc.tile_pool(name="work_ps", bufs=1, space="PSUM") as work_ps,
        ):
            # ---------------- persistent SBUF ----------------
            XT = persist.tile([P, KO, S], F16, name="XT")
            QT3 = persist.tile([P, 3, S], F16, name="QT3")
            KT3 = persist.tile([P, 3, S], F16, name="KT3")
            VA = persist.tile([P, SO, NH * (HD + 1)], F16, name="VA")
            wqh = persist.tile([P, NP, KO, P], F16, name="wqh")
            wkh = persist.tile([P, NP, KO, P], F16, name="wkh")
            wvh = persist.tile([P, KO, H], F16, name="wvh")
            nbias = persist.tile([P, 1], F32, name="nbias")
            bq_s = persist.tile([P, KO], F32, name="bq_s")
            bk_s = persist.tile([P, KO], F32, name="bk_s")
            pmask = persist.tile([P, P], F16, name="pmask")
            w16 = persist.tile([P, 512], F16, name="w16")

            # vector-engine setup (fast, unblocks warmup + exp + penalty)
            nc.vector.memset(w16[:], 1.0)
            nc.vector.memset(nbias[:], -PEN)
            nc.vector.memset(pmask[:], 1.0)
            nc.vector.memset(pmask[0:64, 0:64], FIX)
            nc.vector.memset(pmask[64:128, 64:128], FIX)
            # ones columns of V_aug only; V block copies fill the rest
            VA_v = VA[:].rearrange("p s (h c) -> p s h c", c=HD + 1)
            nc.vector.memset(VA_v[:, :, :, HD : HD + 1], 1.0)

            # ---------------- DMA: prioritized, fp16, contiguous ----------
            # 4 hw DMA queues for the head: sync (SP), scalar (Activation),
            # gpsimd, and vector (only its first-scores inputs; vector's
            # compute role starts later).  X chunks + first q/k slices
            # first; biases and Wv behind; late q/k slices last.
            nc.gpsimd.dma_start(out=bq_s[:], in_=bq)
            nc.gpsimd.dma_start(out=bk_s[:], in_=bk)
            nc.vector.tensor_scalar_mul(bq_s[:], bq_s[:], SCALE)
            nc.sync.dma_start(out=XT[:, 0:3, :], in_=xt[:, 0:3, :])
            nc.scalar.dma_start(out=XT[:, 3:6, :], in_=xt[:, 3:6, :])
            nc.scalar.dma_start(out=wkh[:, 0, :, :], in_=wk[:, 0, :, :])
            nc.gpsimd.dma_start(out=XT[:, 6:8, :], in_=xt[:, 6:8, :])
            nc.sync.dma_start(out=wqh[:, 0, :, :], in_=wq[:, 0, :, :])
            nc.gpsimd.dma_start(out=wqh[:, 1, :, :], in_=wq[:, 1, :, :])
            nc.gpsimd.dma_start(out=wkh[:, 1, :, :], in_=wk[:, 1, :, :])
            nc.scalar.dma_start(out=wvh[:, 0:4, :], in_=wv[:, 0:4, :])
            nc.sync.dma_start(out=wvh[:, 4:8, :], in_=wv[:, 4:8, :])
            for mo in range(2, NP):
                nc.gpsimd.dma_start(out=wqh[:, mo, :, :], in_=wq[:, mo, :, :])
                nc.gpsimd.dma_start(out=wkh[:, mo, :, :], in_=wk[:, mo, :, :])

            # ---------------- PE warm-up ----------------
            # ~48 junk matmuls (N=128, fp16) keep the PE busy from ~7us so
            # the HAM clock-gate is at 8/8 when the first projection lands.
            warm = work_ps.tile([P, 512], F32, name="warm", tag="work")
            for w in range(22):
                nc.tensor.matmul(
                    warm[:], w16[:, 0:P], w16[:],
                    start=(w == 0), stop=(w == 21),
                )

            # probs rings: 2 pairs in flight (scored / being consumed by AV)
            prT = [
                [
                    probs.tile([P, KO, S], F16, name=f"prT_{h}_{r}")
                    for r in range(2)
                ]
                for h in range(2)
            ]
            ctxt_sb = [
                probs.tile([80, 512], F16, name=f"ctxt_sb{r}")
                for r in range(4)
            ]
            tpx_sb = [
                probs.tile([P, 4, 80], F16, name=f"tpx_sb{r}")
                for r in range(4)
            ]
            for r in range(4):
                nc.vector.memset(ctxt_sb[r][HD : 80, :], 0.0)
            inv_sb = [
                probs.tile([P, 4], F32, name=f"inv{r}")
                for r in range(4)
            ]
            out_pair = [
                probs.tile([P, SO, P], F16, name=f"out_pair{r}")
                for r in range(2)
            ]

            # score psum: 3 rotating [128, S] tiles; a score pair only
            # WARs the exp from 2 steps back, so pairs issue concurrently
            pst = [
                score_ps.tile([P, S], F32, name=f"pst{h}", tag=f"pst{h}")
                for h in range(3)
            ]
            pst_ctr = [0]

            # ---------------- emission helpers ----------------
            def proj_burst(mo, which, sc, pool=None):
                """One Q or K projection burst: 8 chained MMs -> 1 psum bank,
                evacuated to the QT/KT ring with scale+bias fused."""
                wsb = wqh if which == "q" else wkh
                r = mo % 3
                pl = pool if pool is not None else proj_ps
                ps = pl.tile([P, 512], F32, name="ps",
                             tag="proj" if pl is proj_ps else "work")
                for kh in range(KO):
                    nc.tensor.matmul(
                        ps[:],
                        wsb[:, mo, kh, :],
                        XT[:, kh, sc * 512 : (sc + 1) * 512],
                        start=(kh == 0),
                        stop=(kh == KO - 1),
                    )
                if which == "q":
                    nc.vector.tensor_scalar(
                        QT3[:, r, sc * 512 : (sc + 1) * 512], ps[:],
                        SCALE, bq_s[:, mo : mo + 1],
                        mybir.AluOpType.mult, mybir.AluOpType.add,
                    )
                else:
                    nc.vector.tensor_scalar_add(
                        KT3[:, r, sc * 512 : (sc + 1) * 512], ps[:],
                        bk_s[:, mo : mo + 1],
                    )

            def v_burst(so, ncol):
                """V projection burst: 8 chained MMs -> work ring bank, then
                strided copy into V_aug (leaving the ones columns alone)."""
                ps = work_ps.tile([P, 512], F32, name="vps", tag="work")
                for kh in range(KO):
                    nc.tensor.matmul(
                        ps[:],
                        XT[:, kh, so * P : (so + 1) * P],
                        wvh[:, kh, ncol * 512 : (ncol + 1) * 512],
                        start=(kh == 0),
                        stop=(kh == KO - 1),
                    )
                va_v = VA[:, so, :].rearrange("p (h c) -> p h c", c=HD + 1)
                nc.vector.tensor_copy(
                    va_v[:, ncol * 8 : (ncol + 1) * 8, 0:HD],
                    ps[:].rearrange("p (h c) -> p h c", c=HD),
                )

            def score_qc(i, kt, qc):
                """One qc's pair of K=64 row-tiled score MMs (concurrent)."""
                r = i % 3
                base = pst_ctr[0]
                for half in range(2):
                    lo = half * 64
                    nc.tensor.matmul(
                        pst[(base + half) % 3][:, qc * 512 : (qc + 1) * 512],
                        KT3[lo : lo + 64, r, kt * P : (kt + 1) * P],
                        QT3[lo : lo + 64, r, qc * 512 : (qc + 1) * 512],
                        start=True,
                        stop=True,
                        tile_position=(lo, 0),
                    )
                if qc == 1:
                    pst_ctr[0] = base + 2

            def exp_kt(i, kt, ring):
                """Per-half exps (pst rotation makes the pairs concurrent)."""
                base = pst_ctr[0] - 2
                for half in range(2):
                    dst = prT[half][ring]
                    nc.scalar.activation(
                        dst[:, kt, :], pst[(base + half) % 3][:], Exp,
                        bias=nbias[:],
                    )
                    nc.vector.tensor_mul(
                        dst[:, kt, kt * P : (kt + 1) * P],
                        dst[:, kt, kt * P : (kt + 1) * P],
                        pmask[:],
                    )

            def av_open(i, half, qc, ring, pool, kt_hi):
                """Open an AV accumulation chain over kt 0..kt_hi-1."""
                ctx = pool.tile([P, 512], F32, name="ctx",
                                tag="proj" if pool is proj_ps else "work")
                return av_open_into(ctx, i, half, qc, ring, kt_hi)

            def av_open_into(ctx, i, half, qc, ring, kt_hi):
                h = 2 * i + half
                for kt in range(kt_hi):
                    nc.tensor.matmul(
                        ctx[0 : HD + 1, :],
                        VA[:, kt, h * (HD + 1) : (h + 1) * (HD + 1)],
                        prT[half][ring][:, kt, qc * 512 : (qc + 1) * 512],
                        start=(kt == 0),
                        stop=False,
                    )
                return ctx

            def av_extend(ctx, i, half, qc, ring, kt_lo, kt_hi):
                h = 2 * i + half
                for kt in range(kt_lo, kt_hi):
                    nc.tensor.matmul(
                        ctx[0 : HD + 1, :],
                        VA[:, kt, h * (HD + 1) : (h + 1) * (HD + 1)],
                        prT[half][ring][:, kt, qc * 512 : (qc + 1) * 512],
                        start=False,
                        stop=False,
                    )

            def av_close(ctx, i, half, qc, ring, slot4, kt_lo, eng_scalar=False):
                """Close an AV chain (kt_lo..7) and evacuate to SBUF."""
                h = 2 * i + half
                for kt in range(kt_lo, KO):
                    nc.tensor.matmul(
                        ctx[0 : HD + 1, :],
                        VA[:, kt, h * (HD + 1) : (h + 1) * (HD + 1)],
                        prT[half][ring][:, kt, qc * 512 : (qc + 1) * 512],
                        start=False,
                        stop=(kt == KO - 1),
                    )
                if eng_scalar:
                    nc.scalar.copy(ctxt_sb[slot4][0 : HD + 1, :], ctx[0 : HD + 1, :])
                else:
                    nc.vector.tensor_copy(
                        ctxt_sb[slot4][0 : HD + 1, :], ctx[0 : HD + 1, :]
                    )

            def av_burst(i, half, qc, ring, slot4):
                """Full AV for (pair i, head-half, q-chunk qc)."""
                ctx = av_open(i, half, qc, ring, work_ps, KO - 1)
                av_close(ctx, i, half, qc, ring, slot4, KO - 1)

            def tp_burst(half, qc, slot4, opr, pool=None, eng=None, ne=None):
                """X-bar DMA-transpose ctx^T back to [q, d] + normalize."""
                tpx = tpx_sb[slot4]
                (eng or nc.sync).dma_start(out=tpx[:], in_=ctxt_sb[slot4][:],
                                           transpose=True)
                ne = ne or nc.vector
                ne.reciprocal(inv_sb[slot4][:], tpx[:, :, HD])
                for c4 in range(4):
                    so = qc * 4 + c4
                    ne.tensor_scalar_mul(
                        out_pair[opr][:, so, half * HD : (half + 1) * HD],
                        tpx[:, c4, 0:HD],
                        inv_sb[slot4][:, c4 : c4 + 1],
                    )

            def ship_pair(i, opr):
                nc.sync.dma_start(
                    out=out_r[:, :, i * P : (i + 1) * P], in_=out_pair[opr][:]
                )

            # ---------------- bootstrap: proj(0) ----------------
            proj_burst(0, "q", 0)
            proj_burst(0, "q", 1, pool=work_ps)
            proj_burst(0, "k", 0)
            proj_burst(0, "k", 1, pool=work_ps)

            # ---------------- pair slots ----------------
            # slot i: scores(i) + exp(i) | proj(i+1) | V bursts (slot 0 +
            # spread) | i>=1: AV(i-1) + tp(i-1) + ship(i-1)
            av_seq = [(0, 0), (0, 1), (1, 0), (1, 1)]
            cA = [None] * 4
            for i in range(NP):
                ring = i % 2
                pring = (i - 1) % 2
                opr = (i - 1) % 2

                for kt in range(KO):
                    # all four score MMs contiguous: each qc's half-pair runs
                    # concurrently (pst rotation), and only one K64<->K128
                    # transition pair per kt (transitions cost ~100ns extra)
                    score_qc(i, kt, 0)
                    score_qc(i, kt, 1)
                    exp_kt(i, kt, ring)

                    if i + 1 < NP:
                        if kt == 1:
                            proj_burst(i + 1, "q", 0)
                        elif kt == 3:
                            proj_burst(i + 1, "q", 1)
                        elif kt == 5:
                            proj_burst(i + 1, "k", 0)
                        elif kt == 7:
                            proj_burst(i + 1, "k", 1)
                    if i == 0:
                        if with_bv:
                            if kt < 4:
                                v_burst(2 * kt, 0)
                                v_burst(2 * kt, 1)
                            else:
                                v_burst(2 * (kt - 4) + 1, 0)
                                v_burst(2 * (kt - 4) + 1, 1)
                        else:
                            # heads 0-7 here; heads 8-15 spread over slots 1-4
                            v_burst(kt, 0)
                    else:
                        if kt % 2 == 0:
                            half, qc = av_seq[kt // 2]
                            av_burst(i - 1, half, qc, pring, kt // 2)
                        elif kt >= 3:
                            half, qc = av_seq[(kt - 3) // 2]
                            tp_burst(half, qc, (kt - 3) // 2, opr)
                        if not with_bv and 1 <= i <= 4 and kt in (1, 5):
                            v_burst(2 * (i - 1) + (0 if kt == 1 else 1), 1)
                    if i == NP - 1:
                        # build pair-7's AV chains progressively so only the
                        # kt=7 closers remain after the final exp
                        if kt == 3:
                            cA[0] = av_open(i, 0, 0, ring, proj_ps, 4)
                        elif kt == 5:
                            av_extend(cA[0], i, 0, 0, ring, 4, 6)
                        elif kt == 7:
                            av_extend(cA[0], i, 0, 0, ring, 6, 7)
                            b7 = (pst_ctr[0] - 2) % 3
                            ft = [t for t in range(3)
                                  if t not in (b7, (b7 + 1) % 3)][0]
                            ps_e = score_ps.tile([P, S], F32, name="ps_e",
                                                 tag=f"pst{ft}")
                            ps_f = score_ps.tile([P, S], F32, name="ps_f",
                                                 tag=f"pst{b7}")
                            cA[1] = av_open(i, 0, 1, ring, work_ps, KO - 1)
                            cA[2] = av_open_into(ps_e[:, 0:512], i, 1, 0,
                                                 ring, KO - 1)
                            cA[3] = av_open_into(ps_f[:, 0:512], i, 1, 1,
                                                 ring, KO - 1)

                if i == 0 and with_bv:
                    # out += bv exactly (softmax rows sum to 1): broadcast bv
                    # across partitions via PE, add into V_aug.
                    ones1 = persist.tile([1, P], F16, name="ones1")
                    nc.vector.memset(ones1[:], 1.0)
                    bv1 = persist.tile([1, H], F16, name="bv1")
                    bv1_32 = persist.tile([1, H], F32, name="bv1_32")
                    nc.sync.dma_start(out=bv1_32[:], in_=bv[None, :])
                    nc.vector.tensor_copy(bv1[:], bv1_32[:])
                    bvb = persist.tile([P, NH * (HD + 1)], F16, name="bvb")
                    nc.vector.memset(bvb[:], 0.0)
                    bvb_v = bvb.rearrange("p (h c) -> p h c", c=HD + 1)
                    for ncol in range(2):
                        psb = proj_ps.tile([P, 512], F32, name="psb", tag="proj")
                        nc.tensor.matmul(
                            psb[:], ones1[:], bv1[:, ncol * 512 : (ncol + 1) * 512],
                            start=True, stop=True,
                        )
                        nc.vector.tensor_copy(
                            bvb_v[:, ncol * 8 : (ncol + 1) * 8, 0:HD],
                            psb[:].rearrange("p (h c) -> p h c", c=HD),
                        )
                    for so in range(SO):
                        nc.vector.tensor_add(VA[:, so, :], VA[:, so, :], bvb[:])

                if i >= 1:
                    # last tp of pair i-1 spills here; then ship the pair
                    tp_burst(1, 1, 3, opr)
                    ship_pair(i - 1, opr)

            # ---------------- epilogue: AV(7) + tp(7), split-chain --------
            # Open all four AV chains over kt 0..6 (their exps are done while
            # kt=7's exps still run); only the kt=7 closers + transposes +
            # ship are exposed after the final exp.
            ring = (NP - 1) % 2
            opr = (NP - 1) % 2
            av_close(cA[0], NP - 1, 0, 0, ring, 0, KO - 1, eng_scalar=True)
            tp_burst(0, 0, 0, opr, eng=nc.sync)
            av_close(cA[1], NP - 1, 0, 1, ring, 1, KO - 1, eng_scalar=True)
            tp_burst(0, 1, 1, opr, eng=nc.scalar)
            av_close(cA[2], NP - 1, 1, 0, ring, 2, KO - 1, eng_scalar=True)
            tp_burst(1, 0, 2, opr, eng=nc.sync)
            av_close(cA[3], NP - 1, 1, 1, ring, 3, KO - 1, eng_scalar=True)
            tp_burst(1, 1, 3, opr, eng=nc.scalar)
            # final ship split by SO halves across two queues
            nc.sync.dma_start(
                out=out_r[:, 0:4, (NP - 1) * P : NP * P],
                in_=out_pair[opr][:, 0:4, :],
            )
            nc.scalar.dma_start(
                out=out_r[:, 4:8, (NP - 1) * P : NP * P],
                in_=out_pair[opr][:, 4:8, :],
            )

    nc.compile()
    return nc


def _get_program(with_bv: bool):
    key = with_bv
    if key not in _programs:
        _programs[key] = _build_program(with_bv)
    return _programs[key]


def _in_maps(hidden_states, Wq, bq, Wk, bk, Wv, bv):
    # fp16 pre-layout matching the SBUF tiling (see _build_program):
    #   xt[kp, ko, s]     = X[s, ko*128+kp]
    #   wq[kp, mo, ko, c] = Wq[ko*128+kp, mo*128+c]   (wk likewise)
    #   wv[kp, ko, n]     = Wv[ko*128+kp, n]
    wq16 = np.ascontiguousarray(
        np.asarray(Wq, np.float32).astype(np.float16)
        .reshape(KO, P, NP, P).transpose(1, 2, 0, 3)
    )
    wk16 = np.ascontiguousarray(
        np.asarray(Wk, np.float32).astype(np.float16)
        .reshape(KO, P, NP, P).transpose(1, 2, 0, 3)
    )
    wv16 = np.ascontiguousarray(
        np.asarray(Wv, np.float32).astype(np.float16)
        .reshape(KO, P, H).transpose(1, 0, 2)
    )
    bq = np.ascontiguousarray(
        np.asarray(bq, np.float32).reshape(KO, P).T)
    bk = np.ascontiguousarray(
        np.asarray(bk, np.float32).reshape(KO, P).T)
    bv = np.ascontiguousarray(bv, np.float32)
    maps = []
    for b in range(B):
        xt16 = np.ascontiguousarray(
            np.asarray(hidden_states[b], np.float32).astype(np.float16)
            .T.reshape(KO, P, S).transpose(1, 0, 2)
        )
        maps.append({
            "xt": xt16,
            "wq": wq16, "wk": wk16, "wv": wv16,
            "bq": bq, "bk": bk, "bv": bv,
        })
    return maps


def kernel(hidden_states, Wq, bq, Wk, bk, Wv, bv):
    hidden_states = np.ascontiguousarray(hidden_states, dtype=np.float32)
    with_bv = bool(np.any(np.asarray(bv) != 0))
    nc = _get_program(with_bv)
    in_maps = _in_maps(hidden_states, Wq, bq, Wk, bk, Wv, bv)
    last_err = None
    for _attempt in range(3):
        try:
            res = run_bass_kernel_spmd(nc, in_maps, list(range(B)))
            return np.stack([res.results[b]["out"] for b in range(B)],
                            axis=0).astype(np.float32)
        except Exception as e:  # transient NRT device errors recover on retry
            last_err = e
            import time
            time.sleep(3)
    raise last_err
